# revision 15
# baseline (speedup 1.0000x reference)
"""Multi-head attention block (qkv proj + softmax attention + out proj) on 8
TRN2 NeuronCores, data-parallel over the batch dimension (2 batches/core).

Reference computation (B=16, N=1024, C=1024, H=16, D=64):
    qkv = x @ w_qkv.T                    # [B,N,3C]
    q,k,v per head; attn = softmax(q k^T / sqrt(D)); out = attn @ v
    out = concat_heads @ w_proj.T + b_proj

Device-side design (per core, T = 2*1024 tokens), all channels-on-partitions
("transposed") so the softmax denominator falls out of the PV matmul:
    qk:   qT/kT[o, t] = sum_c wqkT[c, o] * xT[c, t], evicted straight into
          SBUF pair tiles (no DRAM roundtrip)
    v:    v[t, vc]    = sum_c xT[c, t] * wvT[c, vc]  (+ ones column per head)
    attn: S_T[key, q] = kT.T @ qT  (K=64, the two heads of a pair row-packed
          into halves of one 2-bank PSUM tile), ONE exp over [128, 1024] on
          ACT per key tile (no max subtraction needed: |S*scale| < 6),
          pv[d|1, q] = [v_h | 1].T @ E  accumulated over key tiles
          -> partition 64 of pv is the softmax denominator;
          normalize: denom -> SBUF, reciprocal_approx_fast, GPSIMD
          partition_broadcast, attnT = pv * recip  (fp16)
    out:  out[t, o] = sum_c attnT[c, t] * wpT[c, o] + bias
All matmul inputs fp16 with fp32 PSUM accumulation.

The emission order software-pipelines the whole kernel so the tensor engine
never idles on the ACT exp round-trip (an idle PE re-throttles to half
clock): each attention unit (batch, head-pair) inlines the qk projection of
the unit two ahead, and hook points inside the key-tile loop pop extra
projection / output-projection chains from filler queues. The PV matmuls run
one key tile behind the S matmuls so the exp latency is hidden.
"""

import numpy as np

B, N, C = 16, 1024, 1024
H, D = 16, 64
SCALE = D ** -0.5
NCORES = 8
B_SH = B // NCORES            # batches per core
T = B_SH * N                  # tokens per core
CT = C // 128                 # 128-channel tiles per C
PAIRS = H // 2                # head pairs (2 heads share a 128-row tile)
KT = N // 128                 # key tiles per batch
QB = N // 512                 # q blocks of 512 per batch

_CACHE = {}


def _build():
    import concourse.mybir as mybir
    import concourse.tile as tile
    from concourse import bacc

    fp16 = mybir.dt.float16
    fp32 = mybir.dt.float32
    Exp = mybir.ActivationFunctionType.Exp

    nc = bacc.Bacc("TRN2", target_bir_lowering=False, debug=False)

    xT = nc.dram_tensor("xT", [C, T], fp16, kind="ExternalInput")
    wqkT = nc.dram_tensor("wqkT", [C, 2 * C], fp16, kind="ExternalInput")
    wvT = nc.dram_tensor("wvT", [C, C], fp16, kind="ExternalInput")
    wpT = nc.dram_tensor("wpT", [C, C], fp16, kind="ExternalInput")
    bias = nc.dram_tensor("bias", [128, C], fp32, kind="ExternalInput")
    out = nc.dram_tensor("out", [T, C], fp32, kind="ExternalOutput")

    xT_r = xT.rearrange("(j p) t -> p j t", p=128)
    wqkT_r = wqkT.rearrange("(j p) o -> p j o", p=128)
    wvT_r = wvT.rearrange("(j p) o -> p j o", p=128)
    wpT_r = wpT.rearrange("(j p) o -> p j o", p=128)

    with tile.TileContext(nc) as tc:
        with (
            tc.tile_pool(name="persist", bufs=1) as persist,
            tc.tile_pool(name="wstream", bufs=4) as wstream,
            tc.tile_pool(name="qk2", bufs=4) as qk2,
            tc.tile_pool(name="epool", bufs=4) as epool,
            tc.tile_pool(name="dnpool", bufs=4) as dnpool,
            tc.tile_pool(name="bcpool", bufs=4) as bcpool,
            tc.tile_pool(name="outpool", bufs=4) as outpool,
            tc.tile_pool(name="psum", bufs=1, space="PSUM") as psum,
        ):
            # [tok%128, tok//128, head, d | ones]
            v_ext = persist.tile([128, 2 * KT, H, D + 1], fp16)
            # [c%128, c//128, tok]
            attnT = persist.tile([128, CT, T], fp16)
            x_sb = [persist.tile([128, T], fp16, name=f"x{ct}")
                    for ct in range(CT)]
            wv_sb = persist.tile([128, CT, C], fp16)
            wp_sb = persist.tile([128, CT, C], fp16)
            bias_sb = persist.tile([128, C], fp32)
            zero_sb = persist.tile([128, 1], fp32)

            # ---------------- work units ----------------
            def wq_load(ot, stage):
                wq_sb = wstream.tile([128, CT, 128], fp16, tag="wq",
                                     name=f"wq_{ot}_{stage}")
                nc.sync.dma_start(
                    wq_sb[:], wqkT_r[:, :, ot * 128:(ot + 1) * 128])
                return wq_sb

            def qk_pair_unit(b, p, wqs):
                """Project q and k channels of pair p for batch b's tokens,
                evicting straight into SBUF tiles shaped [128, N]."""
                q_sb = qk2.tile([128, N], fp16, tag="q", name=f"q{b}_{p}")
                k_sb = qk2.tile([128, N], fp16, tag="k", name=f"k{b}_{p}")
                for wq_sb, dst in zip(wqs, (q_sb, k_sb)):
                    for half in range(2):
                        tb = 2 * b + half
                        ps = psum.tile([128, 512], fp32, tag="mm", bufs=2,
                                       name=f"qkps_{b}_{p}_{half}")
                        for ct in range(CT):
                            nc.tensor.matmul(
                                ps[:], wq_sb[:, ct, :],
                                x_sb[ct][:, tb * 512:(tb + 1) * 512],
                                start=(ct == 0), stop=(ct == CT - 1))
                        nc.vector.tensor_copy(
                            dst[:, half * 512:(half + 1) * 512], ps[:])
                return q_sb, k_sb

            def v_chain(tt, ob):
                ps = psum.tile([128, 512], fp32, tag="mm", bufs=2,
                               name=f"vps_{tt}_{ob}")
                for ct in range(CT):
                    nc.tensor.matmul(
                        ps[:], x_sb[ct][:, tt * 128:(tt + 1) * 128],
                        wv_sb[:, ct, ob * 512:(ob + 1) * 512],
                        start=(ct == 0), stop=(ct == CT - 1))
                nc.vector.tensor_copy(
                    v_ext[:, tt, ob * 8:(ob + 1) * 8, 0:D],
                    ps[:].rearrange("p (h d) -> p h d", d=D))

            def out_chain(tt, ob):
                ps = psum.tile([128, 512], fp32, tag="mm", bufs=2,
                               name=f"ops_{tt}_{ob}")
                for j in range(CT):
                    nc.tensor.matmul(
                        ps[:], attnT[:, j, tt * 128:(tt + 1) * 128],
                        wp_sb[:, j, ob * 512:(ob + 1) * 512],
                        start=(j == 0), stop=(j == CT - 1))
                o_sb = outpool.tile([128, 512], fp32, tag="o",
                                    name=f"osb_{tt}_{ob}")
                nc.vector.tensor_add(
                    o_sb[:], ps[:], bias_sb[:, ob * 512:(ob + 1) * 512])
                nc.sync.dma_start(
                    out[tt * 128:(tt + 1) * 128, ob * 512:(ob + 1) * 512],
                    o_sb[:])

            def attn_unit(b, p, q_sb, k_sb, fillers, max_fill):
                filled = [0]

                def maybe_fill():
                    if fillers and filled[0] < max_fill:
                        fillers.pop()
                        filled[0] += 1

                for qb in range(QB):
                    pv = [psum.tile([D + 1, 512], fp32, tag="pv", bufs=2,
                                    name=f"pv_{b}_{p}_{qb}_{h2}")
                          for h2 in range(2)]
                    e_prev = None
                    for kt in range(KT):
                        # both heads' scores into halves of one 2-bank tile,
                        # one exp over the whole [128, 1024] region
                        s_ps = psum.tile([128, 1024], fp32, tag="s", bufs=2,
                                         name=f"s_{b}_{p}_{qb}_{kt}")
                        for h2 in range(2):
                            ho = h2 * 64
                            nc.tensor.matmul(
                                s_ps[:, h2 * 512:(h2 + 1) * 512],
                                k_sb[ho:ho + 64, kt * 128:(kt + 1) * 128],
                                q_sb[ho:ho + 64, qb * 512:(qb + 1) * 512],
                                start=True, stop=True)
                        e_sb = epool.tile([128, 1024], fp16, tag="e",
                                          name=f"e_{b}_{p}_{qb}_{kt}")
                        nc.scalar.activation(
                            e_sb[:], s_ps[:], Exp,
                            bias=zero_sb[:], scale=SCALE)
                        if kt > 0:
                            for h2 in range(2):
                                nc.tensor.matmul(
                                    pv[h2][:],
                                    v_ext[:, b * KT + kt - 1, 2 * p + h2, :],
                                    e_prev[:, h2 * 512:(h2 + 1) * 512],
                                    start=(kt == 1), stop=False)
                        e_prev = e_sb
                        if kt in (2, 5):
                            maybe_fill()
                    for h2 in range(2):
                        nc.tensor.matmul(
                            pv[h2][:],
                            v_ext[:, b * KT + KT - 1, 2 * p + h2, :],
                            e_prev[:, h2 * 512:(h2 + 1) * 512],
                            start=False, stop=True)
                    for h2 in range(2):
                        dn_raw = dnpool.tile([1, 512], fp32, tag="dnr",
                                             name=f"dnr_{b}_{p}_{qb}_{h2}")
                        nc.vector.tensor_copy(dn_raw[0:1, :], pv[h2][D:D + 1, :])
                        dn = dnpool.tile([1, 512], fp32, tag="dn",
                                         name=f"dn_{b}_{p}_{qb}_{h2}")
                        nc.vector.reciprocal_approx_fast(
                            dn[0:1, :], dn_raw[0:1, :])
                        bc_sb = bcpool.tile([64, 512], fp32, tag="bc",
                                            name=f"bc_{b}_{p}_{qb}_{h2}")
                        nc.gpsimd.partition_broadcast(
                            bc_sb[:], dn[0:1, :], channels=64)
                        tsl = slice(b * N + qb * 512, b * N + (qb + 1) * 512)
                        nc.vector.tensor_mul(
                            attnT[h2 * 64:h2 * 64 + 64, p, tsl],
                            pv[h2][0:D, :], bc_sb[:])
                    maybe_fill()

            # -------------------- emission schedule -----------------------
            class FillerQueue:
                def __init__(self, items):
                    self.items = list(items)
                    self.i = 0

                def pop(self):
                    if self.i < len(self.items):
                        self.items[self.i]()
                        self.i += 1

                def __bool__(self):
                    return self.i < len(self.items)

                def flush(self):
                    while self:
                        self.pop()

            units = [(0, p) for p in range(PAIRS)] + \
                    [(1, p) for p in range(PAIRS)]

            # prologue: weights for the first two pair projections, then x
            wq_tiles = {0: [wq_load(units[0][1], "a0"),
                            wq_load(CT + units[0][1], "a1")]}
            for ct in range(CT):
                nc.sync.dma_start(x_sb[ct][:], xT_r[:, ct, :])
            wq_tiles[1] = [wq_load(units[1][1], "b0"),
                           wq_load(CT + units[1][1], "b1")]
            nc.sync.dma_start(wv_sb[:], wvT_r)
            nc.sync.dma_start(wp_sb[:], wpT_r)
            nc.sync.dma_start(bias_sb[:], bias[:])
            nc.any.memset(zero_sb[:], 0.0)
            nc.any.memset(v_ext[:, :, :, D:D + 1], 1.0)

            # projections for the first two units + batch-0 v chains
            qk_ready = {}
            wq_tiles[2] = [wq_load(units[2][1], "c0"),
                           wq_load(CT + units[2][1], "c1")]
            qk_ready[0] = qk_pair_unit(*units[0], wq_tiles.pop(0))
            wq_tiles[3] = [wq_load(units[3][1], "d0"),
                           wq_load(CT + units[3][1], "d1")]
            qk_ready[1] = qk_pair_unit(*units[1], wq_tiles.pop(1))
            for tt in range(KT):
                for ob in range(2):
                    v_chain(tt, ob)

            fq_b = FillerQueue([lambda tt=tt, ob=ob: v_chain(tt, ob)
                                for tt in range(KT, 2 * KT) for ob in range(2)])
            fq_c = FillerQueue([lambda tt=tt, ob=ob: out_chain(tt, ob)
                                for tt in range(KT) for ob in range(2)])

            for i, (b, p) in enumerate(units):
                # prefetch weights for the pair projection emitted next unit
                if i + 4 < len(units):
                    pq = units[i + 4][1]
                    wq_tiles[i + 4] = [wq_load(pq, f"u{i}a"),
                                       wq_load(CT + pq, f"u{i}b")]
                # inline projection for the unit two ahead
                if i + 2 < len(units):
                    qk_ready[i + 2] = qk_pair_unit(*units[i + 2],
                                                   wq_tiles.pop(i + 2))
                q_sb, k_sb = qk_ready.pop(i)
                if b == 0:
                    attn_unit(b, p, q_sb, k_sb, fq_b, max_fill=3)
                else:
                    attn_unit(b, p, q_sb, k_sb, fq_c, max_fill=3)
                if i == PAIRS - 1:
                    fq_b.flush()
            fq_c.flush()

            # batch-1 output projection tail
            for tt in range(KT, 2 * KT):
                for ob in range(2):
                    out_chain(tt, ob)

    nc.compile()
    return nc


def _get_nc():
    if "nc" not in _CACHE:
        _CACHE["nc"] = _build()
    return _CACHE["nc"]


def _prep_inputs(x, w_qkv, w_proj, b_proj):
    x16 = np.ascontiguousarray(x, dtype=np.float16)
    wq16 = np.asarray(w_qkv, dtype=np.float16)
    wp16 = np.asarray(w_proj, dtype=np.float16)
    wqkT_np = np.ascontiguousarray(wq16[0:2 * C].T)          # [C, 2C]
    wvT_np = np.ascontiguousarray(wq16[2 * C:3 * C].T)       # [C, C]
    wpT_np = np.ascontiguousarray(wp16.T)                    # [C, C]
    bias_np = np.ascontiguousarray(
        np.broadcast_to(np.asarray(b_proj, dtype=np.float32)[None, :], (128, C)))
    in_maps = []
    for core in range(NCORES):
        xs = x16[core * B_SH:(core + 1) * B_SH]              # [B_SH, N, C]
        xT_np = np.ascontiguousarray(xs.transpose(2, 0, 1).reshape(C, T))
        in_maps.append({
            "xT": xT_np, "wqkT": wqkT_np, "wvT": wvT_np,
            "wpT": wpT_np, "bias": bias_np,
        })
    return in_maps


def _install_ntff_hook():
    """The agent image's antenv lacks axon_hooks; synthesize it so
    run_bass_kernel_spmd(trace=True) can capture NTFF profiles."""
    import sys
    import types
    try:
        from antenv.axon_hooks import get_axon_ntff_profile_hook  # noqa: F401
        return
    except ImportError:
        pass
    import antenv
    mod = types.ModuleType("antenv.axon_hooks")
    state = {"hook": None}
    mod.set_axon_ntff_profile_hook = lambda h: state.__setitem__("hook", h)
    mod.get_axon_ntff_profile_hook = lambda: state["hook"]
    sys.modules["antenv.axon_hooks"] = mod
    antenv.axon_hooks = mod
    try:
        from trn_agent_boot.trn_boot import _ntff_profile_via_ctypes
        mod.set_axon_ntff_profile_hook(
            _ntff_profile_via_ctypes("/opt/axon/libaxon_pjrt.so"))
    except Exception as e:  # tracing degrades, run still works
        print("ntff hook install failed:", e)


def run(x, w_qkv, w_proj, b_proj, trace=False):
    """Returns (full_output [B,N,C] fp32, BassKernelResults)."""
    from concourse.bass_utils import run_bass_kernel_spmd

    if trace:
        _install_ntff_hook()
    nc = _get_nc()
    in_maps = _prep_inputs(x, w_qkv, w_proj, b_proj)
    res = run_bass_kernel_spmd(
        nc, in_maps, core_ids=list(range(NCORES)), trace=trace)
    out_full = np.concatenate(
        [r["out"].reshape(B_SH, N, C) for r in res.results], axis=0)
    return out_full.astype(np.float32), res


def kernel(x, w_qkv, w_proj, b_proj):
    out_full, _ = run(x, w_qkv, w_proj, b_proj, trace=False)
    return out_full


# revision 16
# speedup vs baseline: 1.0294x; 1.0294x over previous
"""Multi-head attention block (qkv proj + softmax attention + out proj) on 8
TRN2 NeuronCores, data-parallel over the batch dimension (2 batches/core).

Reference computation (B=16, N=1024, C=1024, H=16, D=64):
    qkv = x @ w_qkv.T                    # [B,N,3C]
    q,k,v per head; attn = softmax(q k^T / sqrt(D)); out = attn @ v
    out = concat_heads @ w_proj.T + b_proj

Device-side design (per core, T = 2*1024 tokens), all channels-on-partitions
("transposed") so the softmax denominator falls out of the PV matmul:
    qk:   qT/kT[o, t] = sum_c wqkT[c, o] * xT[c, t], evicted straight into
          SBUF pair tiles (no DRAM roundtrip)
    v:    v[t, vc]    = sum_c xT[c, t] * wvT[c, vc]  (+ ones column per head)
    attn: S_T[key, q] = kT.T @ qT  (K=64, the two heads of a pair row-packed
          into halves of one 2-bank PSUM tile), ONE exp over [128, 1024] on
          ACT per key tile (no max subtraction needed: |S*scale| < 6),
          pv[d|1, q] = [v_h | 1].T @ E  accumulated over key tiles
          -> partition 64 of pv is the softmax denominator;
          normalize: denom -> SBUF, reciprocal_approx_fast, GPSIMD
          partition_broadcast, attnT = pv * recip  (fp16)
    out:  out[t, o] = sum_c attnT[c, t] * wpT[c, o] + bias
All matmul inputs fp16 with fp32 PSUM accumulation.

The emission order software-pipelines the whole kernel so the tensor engine
never idles on the ACT exp round-trip (an idle PE re-throttles to half
clock): each attention unit (batch, head-pair) inlines the qk projection of
the unit two ahead, and hook points inside the key-tile loop pop extra
projection / output-projection chains from filler queues. The PV matmuls run
one key tile behind the S matmuls so the exp latency is hidden.
"""

import numpy as np

B, N, C = 16, 1024, 1024
H, D = 16, 64
SCALE = D ** -0.5
NCORES = 8
B_SH = B // NCORES            # batches per core
T = B_SH * N                  # tokens per core
CT = C // 128                 # 128-channel tiles per C
PAIRS = H // 2                # head pairs (2 heads share a 128-row tile)
KT = N // 128                 # key tiles per batch
QB = N // 512                 # q blocks of 512 per batch

_CACHE = {}


def _build():
    import concourse.mybir as mybir
    import concourse.tile as tile
    from concourse import bacc

    fp16 = mybir.dt.float16
    fp32 = mybir.dt.float32
    Exp = mybir.ActivationFunctionType.Exp

    nc = bacc.Bacc("TRN2", target_bir_lowering=False, debug=False)

    xT = nc.dram_tensor("xT", [C, T], fp16, kind="ExternalInput")
    wqkT = nc.dram_tensor("wqkT", [C, 2 * C], fp16, kind="ExternalInput")
    wvT = nc.dram_tensor("wvT", [C, C], fp16, kind="ExternalInput")
    wpT = nc.dram_tensor("wpT", [C, C], fp16, kind="ExternalInput")
    bias = nc.dram_tensor("bias", [128, C], fp32, kind="ExternalInput")
    out = nc.dram_tensor("out", [T, C], fp32, kind="ExternalOutput")

    xT_r = xT.rearrange("(j p) t -> p j t", p=128)
    wqkT_r = wqkT.rearrange("(j p) o -> p j o", p=128)
    wvT_r = wvT.rearrange("(j p) o -> p j o", p=128)
    wpT_r = wpT.rearrange("(j p) o -> p j o", p=128)

    with tile.TileContext(nc) as tc:
        with (
            tc.tile_pool(name="persist", bufs=1) as persist,
            tc.tile_pool(name="wstream", bufs=4) as wstream,
            tc.tile_pool(name="qk2", bufs=4) as qk2,
            tc.tile_pool(name="epool", bufs=4) as epool,
            tc.tile_pool(name="dnpool", bufs=4) as dnpool,
            tc.tile_pool(name="bcpool", bufs=4) as bcpool,
            tc.tile_pool(name="outpool", bufs=4) as outpool,
            tc.tile_pool(name="psum", bufs=1, space="PSUM") as psum,
        ):
            # [tok%128, tok//128, head, d | ones]
            v_ext = persist.tile([128, 2 * KT, H, D + 1], fp16)
            # [c%128, c//128, tok]
            attnT = persist.tile([128, CT, T], fp16)
            x_sb = [persist.tile([128, T], fp16, name=f"x{ct}")
                    for ct in range(CT)]
            wv_sb = persist.tile([128, CT, C], fp16)
            wp_sb = persist.tile([128, CT, C], fp16)
            bias_sb = persist.tile([128, C], fp32)
            zero_sb = persist.tile([128, 1], fp32)

            # ---------------- work units ----------------
            def wq_load(ot, stage):
                wq_sb = wstream.tile([128, CT, 128], fp16, tag="wq",
                                     name=f"wq_{ot}_{stage}")
                nc.sync.dma_start(
                    wq_sb[:], wqkT_r[:, :, ot * 128:(ot + 1) * 128])
                return wq_sb

            def qk_pair_unit(b, p, wqs):
                """Project q and k channels of pair p for batch b's tokens,
                evicting straight into SBUF tiles shaped [128, N]."""
                q_sb = qk2.tile([128, N], fp16, tag="q", name=f"q{b}_{p}")
                k_sb = qk2.tile([128, N], fp16, tag="k", name=f"k{b}_{p}")
                for wq_sb, dst in zip(wqs, (q_sb, k_sb)):
                    for half in range(2):
                        tb = 2 * b + half
                        ps = psum.tile([128, 512], fp32, tag="mm", bufs=2,
                                       name=f"qkps_{b}_{p}_{half}")
                        for ct in range(CT):
                            nc.tensor.matmul(
                                ps[:], wq_sb[:, ct, :],
                                x_sb[ct][:, tb * 512:(tb + 1) * 512],
                                start=(ct == 0), stop=(ct == CT - 1))
                        nc.vector.tensor_copy(
                            dst[:, half * 512:(half + 1) * 512], ps[:])
                return q_sb, k_sb

            def v_chain(tt, ob):
                ps = psum.tile([128, 512], fp32, tag="mm", bufs=2,
                               name=f"vps_{tt}_{ob}")
                for ct in range(CT):
                    nc.tensor.matmul(
                        ps[:], x_sb[ct][:, tt * 128:(tt + 1) * 128],
                        wv_sb[:, ct, ob * 512:(ob + 1) * 512],
                        start=(ct == 0), stop=(ct == CT - 1))
                nc.vector.tensor_copy(
                    v_ext[:, tt, ob * 8:(ob + 1) * 8, 0:D],
                    ps[:].rearrange("p (h d) -> p h d", d=D))

            def out_chain(tt, ob):
                ps = psum.tile([128, 512], fp32, tag="mm", bufs=2,
                               name=f"ops_{tt}_{ob}")
                for j in range(CT):
                    nc.tensor.matmul(
                        ps[:], attnT[:, j, tt * 128:(tt + 1) * 128],
                        wp_sb[:, j, ob * 512:(ob + 1) * 512],
                        start=(j == 0), stop=(j == CT - 1))
                o_sb = outpool.tile([128, 512], fp32, tag="o",
                                    name=f"osb_{tt}_{ob}")
                nc.vector.tensor_add(
                    o_sb[:], ps[:], bias_sb[:, ob * 512:(ob + 1) * 512])
                nc.sync.dma_start(
                    out[tt * 128:(tt + 1) * 128, ob * 512:(ob + 1) * 512],
                    o_sb[:])

            def attn_unit(b, p, q_sb, k_sb, fillers, max_fill):
                filled = [0]

                def maybe_fill():
                    if fillers and filled[0] < max_fill:
                        fillers.pop()
                        filled[0] += 1

                for qb in range(QB):
                    pv = [psum.tile([D + 1, 512], fp32, tag="pv", bufs=2,
                                    name=f"pv_{b}_{p}_{qb}_{h2}")
                          for h2 in range(2)]
                    e_prev = None
                    for kt in range(KT):
                        # both heads' scores into halves of one 2-bank tile,
                        # one exp over the whole [128, 1024] region
                        s_ps = psum.tile([128, 1024], fp32, tag="s", bufs=2,
                                         name=f"s_{b}_{p}_{qb}_{kt}")
                        for h2 in range(2):
                            ho = h2 * 64
                            nc.tensor.matmul(
                                s_ps[:, h2 * 512:(h2 + 1) * 512],
                                k_sb[ho:ho + 64, kt * 128:(kt + 1) * 128],
                                q_sb[ho:ho + 64, qb * 512:(qb + 1) * 512],
                                start=True, stop=True)
                        e_sb = epool.tile([128, 1024], fp16, tag="e",
                                          name=f"e_{b}_{p}_{qb}_{kt}")
                        nc.scalar.activation(
                            e_sb[:], s_ps[:], Exp,
                            bias=zero_sb[:], scale=SCALE)
                        if kt > 0:
                            for h2 in range(2):
                                nc.tensor.matmul(
                                    pv[h2][:],
                                    v_ext[:, b * KT + kt - 1, 2 * p + h2, :],
                                    e_prev[:, h2 * 512:(h2 + 1) * 512],
                                    start=(kt == 1), stop=False)
                        e_prev = e_sb
                        if kt in (1, 4):
                            maybe_fill()
                    for h2 in range(2):
                        nc.tensor.matmul(
                            pv[h2][:],
                            v_ext[:, b * KT + KT - 1, 2 * p + h2, :],
                            e_prev[:, h2 * 512:(h2 + 1) * 512],
                            start=False, stop=True)
                    for h2 in range(2):
                        dn_raw = dnpool.tile([1, 512], fp32, tag="dnr",
                                             name=f"dnr_{b}_{p}_{qb}_{h2}")
                        nc.vector.tensor_copy(dn_raw[0:1, :], pv[h2][D:D + 1, :])
                        dn = dnpool.tile([1, 512], fp32, tag="dn",
                                         name=f"dn_{b}_{p}_{qb}_{h2}")
                        nc.vector.reciprocal_approx_fast(
                            dn[0:1, :], dn_raw[0:1, :])
                        bc_sb = bcpool.tile([64, 512], fp32, tag="bc",
                                            name=f"bc_{b}_{p}_{qb}_{h2}")
                        nc.gpsimd.partition_broadcast(
                            bc_sb[:], dn[0:1, :], channels=64)
                        tsl = slice(b * N + qb * 512, b * N + (qb + 1) * 512)
                        nc.vector.tensor_mul(
                            attnT[h2 * 64:h2 * 64 + 64, p, tsl],
                            pv[h2][0:D, :], bc_sb[:])
                    maybe_fill()

            # -------------------- emission schedule -----------------------
            class FillerQueue:
                def __init__(self, items):
                    self.items = list(items)
                    self.i = 0

                def pop(self):
                    if self.i < len(self.items):
                        self.items[self.i]()
                        self.i += 1

                def __bool__(self):
                    return self.i < len(self.items)

                def flush(self):
                    while self:
                        self.pop()

            units = [(0, p) for p in range(PAIRS)] + \
                    [(1, p) for p in range(PAIRS)]

            # prologue: weights for the first two pair projections, then x
            wq_tiles = {0: [wq_load(units[0][1], "a0"),
                            wq_load(CT + units[0][1], "a1")]}
            for ct in range(CT):
                nc.sync.dma_start(x_sb[ct][:], xT_r[:, ct, :])
            wq_tiles[1] = [wq_load(units[1][1], "b0"),
                           wq_load(CT + units[1][1], "b1")]
            nc.sync.dma_start(wv_sb[:], wvT_r)
            nc.sync.dma_start(wp_sb[:], wpT_r)
            nc.sync.dma_start(bias_sb[:], bias[:])
            nc.any.memset(zero_sb[:], 0.0)
            nc.any.memset(v_ext[:, :, :, D:D + 1], 1.0)

            # projections for the first two units + batch-0 v chains
            qk_ready = {}
            wq_tiles[2] = [wq_load(units[2][1], "c0"),
                           wq_load(CT + units[2][1], "c1")]
            qk_ready[0] = qk_pair_unit(*units[0], wq_tiles.pop(0))
            wq_tiles[3] = [wq_load(units[3][1], "d0"),
                           wq_load(CT + units[3][1], "d1")]
            qk_ready[1] = qk_pair_unit(*units[1], wq_tiles.pop(1))
            for tt in range(KT):
                for ob in range(2):
                    v_chain(tt, ob)

            fq_b = FillerQueue([lambda tt=tt, ob=ob: v_chain(tt, ob)
                                for tt in range(KT, 2 * KT) for ob in range(2)])
            fq_c = FillerQueue([lambda tt=tt, ob=ob: out_chain(tt, ob)
                                for tt in range(KT) for ob in range(2)])

            for i, (b, p) in enumerate(units):
                # prefetch weights for the pair projection emitted next unit
                if i + 4 < len(units):
                    pq = units[i + 4][1]
                    wq_tiles[i + 4] = [wq_load(pq, f"u{i}a"),
                                       wq_load(CT + pq, f"u{i}b")]
                # inline projection for the unit two ahead
                if i + 2 < len(units):
                    qk_ready[i + 2] = qk_pair_unit(*units[i + 2],
                                                   wq_tiles.pop(i + 2))
                q_sb, k_sb = qk_ready.pop(i)
                if b == 0:
                    attn_unit(b, p, q_sb, k_sb, fq_b, max_fill=2)
                else:
                    attn_unit(b, p, q_sb, k_sb, fq_c, max_fill=2)
                if i == PAIRS - 1:
                    fq_b.flush()
            fq_c.flush()

            # batch-1 output projection tail
            for tt in range(KT, 2 * KT):
                for ob in range(2):
                    out_chain(tt, ob)

    nc.compile()
    return nc


def _get_nc():
    if "nc" not in _CACHE:
        _CACHE["nc"] = _build()
    return _CACHE["nc"]


def _prep_inputs(x, w_qkv, w_proj, b_proj):
    x16 = np.ascontiguousarray(x, dtype=np.float16)
    wq16 = np.asarray(w_qkv, dtype=np.float16)
    wp16 = np.asarray(w_proj, dtype=np.float16)
    wqkT_np = np.ascontiguousarray(wq16[0:2 * C].T)          # [C, 2C]
    wvT_np = np.ascontiguousarray(wq16[2 * C:3 * C].T)       # [C, C]
    wpT_np = np.ascontiguousarray(wp16.T)                    # [C, C]
    bias_np = np.ascontiguousarray(
        np.broadcast_to(np.asarray(b_proj, dtype=np.float32)[None, :], (128, C)))
    in_maps = []
    for core in range(NCORES):
        xs = x16[core * B_SH:(core + 1) * B_SH]              # [B_SH, N, C]
        xT_np = np.ascontiguousarray(xs.transpose(2, 0, 1).reshape(C, T))
        in_maps.append({
            "xT": xT_np, "wqkT": wqkT_np, "wvT": wvT_np,
            "wpT": wpT_np, "bias": bias_np,
        })
    return in_maps


def _install_ntff_hook():
    """The agent image's antenv lacks axon_hooks; synthesize it so
    run_bass_kernel_spmd(trace=True) can capture NTFF profiles."""
    import sys
    import types
    try:
        from antenv.axon_hooks import get_axon_ntff_profile_hook  # noqa: F401
        return
    except ImportError:
        pass
    import antenv
    mod = types.ModuleType("antenv.axon_hooks")
    state = {"hook": None}
    mod.set_axon_ntff_profile_hook = lambda h: state.__setitem__("hook", h)
    mod.get_axon_ntff_profile_hook = lambda: state["hook"]
    sys.modules["antenv.axon_hooks"] = mod
    antenv.axon_hooks = mod
    try:
        from trn_agent_boot.trn_boot import _ntff_profile_via_ctypes
        mod.set_axon_ntff_profile_hook(
            _ntff_profile_via_ctypes("/opt/axon/libaxon_pjrt.so"))
    except Exception as e:  # tracing degrades, run still works
        print("ntff hook install failed:", e)


def run(x, w_qkv, w_proj, b_proj, trace=False):
    """Returns (full_output [B,N,C] fp32, BassKernelResults)."""
    from concourse.bass_utils import run_bass_kernel_spmd

    if trace:
        _install_ntff_hook()
    nc = _get_nc()
    in_maps = _prep_inputs(x, w_qkv, w_proj, b_proj)
    res = run_bass_kernel_spmd(
        nc, in_maps, core_ids=list(range(NCORES)), trace=trace)
    out_full = np.concatenate(
        [r["out"].reshape(B_SH, N, C) for r in res.results], axis=0)
    return out_full.astype(np.float32), res


def kernel(x, w_qkv, w_proj, b_proj):
    out_full, _ = run(x, w_qkv, w_proj, b_proj, trace=False)
    return out_full


# revision 20
# speedup vs baseline: 1.0325x; 1.0030x over previous
"""Multi-head attention block (qkv proj + softmax attention + out proj) on 8
TRN2 NeuronCores, data-parallel over the batch dimension (2 batches/core).

Reference computation (B=16, N=1024, C=1024, H=16, D=64):
    qkv = x @ w_qkv.T                    # [B,N,3C]
    q,k,v per head; attn = softmax(q k^T / sqrt(D)); out = attn @ v
    out = concat_heads @ w_proj.T + b_proj

Device-side design (per core, T = 2*1024 tokens), all channels-on-partitions
("transposed") so the softmax denominator falls out of the PV matmul:
    qk:   qT/kT[o, t] = sum_c wqkT[c, o] * xT[c, t], evicted straight into
          SBUF pair tiles (no DRAM roundtrip)
    v:    v[t, vc]    = sum_c xT[c, t] * wvT[c, vc]  (+ ones column per head)
    attn: S_T[key, q] = kT.T @ qT  (K=64, the two heads of a pair row-packed
          into halves of one 2-bank PSUM tile), ONE exp over [128, 1024] on
          ACT per key tile (no max subtraction needed: |S*scale| < 6),
          pv[d|1, q] = [v_h | 1].T @ E  accumulated over key tiles
          -> partition 64 of pv is the softmax denominator;
          normalize: denom -> SBUF, reciprocal_approx_fast, GPSIMD
          partition_broadcast, attnT = pv * recip  (fp16)
    out:  out[t, o] = sum_c attnT[c, t] * wpT[c, o] + bias
All matmul inputs fp16 with fp32 PSUM accumulation.

The emission order software-pipelines the whole kernel so the tensor engine
never idles on the ACT exp round-trip (an idle PE re-throttles to half
clock): each attention unit (batch, head-pair) inlines the qk projection of
the unit two ahead, and hook points inside the key-tile loop pop extra
projection / output-projection chains from filler queues. The PV matmuls run
one key tile behind the S matmuls so the exp latency is hidden.
"""

import numpy as np

B, N, C = 16, 1024, 1024
H, D = 16, 64
SCALE = D ** -0.5
NCORES = 8
B_SH = B // NCORES            # batches per core
T = B_SH * N                  # tokens per core
CT = C // 128                 # 128-channel tiles per C
PAIRS = H // 2                # head pairs (2 heads share a 128-row tile)
KT = N // 128                 # key tiles per batch
QB = N // 512                 # q blocks of 512 per batch

_CACHE = {}


def _build():
    import concourse.mybir as mybir
    import concourse.tile as tile
    from concourse import bacc

    fp16 = mybir.dt.float16
    fp32 = mybir.dt.float32
    Exp = mybir.ActivationFunctionType.Exp

    nc = bacc.Bacc("TRN2", target_bir_lowering=False, debug=False)

    xT = nc.dram_tensor("xT", [C, T], fp16, kind="ExternalInput")
    wqkT = nc.dram_tensor("wqkT", [C, 2 * C], fp16, kind="ExternalInput")
    wvT = nc.dram_tensor("wvT", [C, C], fp16, kind="ExternalInput")
    wpT = nc.dram_tensor("wpT", [C, C], fp16, kind="ExternalInput")
    bias = nc.dram_tensor("bias", [128, C], fp32, kind="ExternalInput")
    out = nc.dram_tensor("out", [T, C], fp32, kind="ExternalOutput")

    xT_r = xT.rearrange("(j p) t -> p j t", p=128)
    wqkT_r = wqkT.rearrange("(j p) o -> p j o", p=128)
    wvT_r = wvT.rearrange("(j p) o -> p j o", p=128)
    wpT_r = wpT.rearrange("(j p) o -> p j o", p=128)

    with tile.TileContext(nc) as tc:
        with (
            tc.tile_pool(name="persist", bufs=1) as persist,
            tc.tile_pool(name="wstream", bufs=4) as wstream,
            tc.tile_pool(name="qk2", bufs=4) as qk2,
            tc.tile_pool(name="epool", bufs=4) as epool,
            tc.tile_pool(name="dnpool", bufs=4) as dnpool,
            tc.tile_pool(name="bcpool", bufs=4) as bcpool,
            tc.tile_pool(name="outpool", bufs=4) as outpool,
            tc.tile_pool(name="psum", bufs=1, space="PSUM") as psum,
        ):
            # [tok%128, tok//128, head, d | ones]
            v_ext = persist.tile([128, 2 * KT, H, D + 1], fp16)
            # [c%128, c//128, tok]
            attnT = persist.tile([128, CT, T], fp16)
            x_sb = [persist.tile([128, T], fp16, name=f"x{ct}")
                    for ct in range(CT)]
            wv_sb = persist.tile([128, CT, C], fp16)
            wp_sb = persist.tile([128, CT, C], fp16)
            bias_sb = persist.tile([128, C], fp32)
            zero_sb = persist.tile([128, 1], fp32)

            # ---------------- work units ----------------
            def wq_load(ot, stage):
                wq_sb = wstream.tile([128, CT, 128], fp16, tag="wq",
                                     name=f"wq_{ot}_{stage}")
                nc.sync.dma_start(
                    wq_sb[:], wqkT_r[:, :, ot * 128:(ot + 1) * 128])
                return wq_sb

            qk_ready = {}
            wq_tiles = {}

            def gen_proj_pair(i):
                """Generator: project q and k channels of pair units[i],
                evicting straight into SBUF tiles shaped [128, N]. Yields
                every ~3 matmuls so attention work interleaves finely."""
                b, p = units[i]
                q_sb = qk2.tile([128, N], fp16, tag="q", name=f"q{b}_{p}")
                k_sb = qk2.tile([128, N], fp16, tag="k", name=f"k{b}_{p}")
                qk_ready[i] = (q_sb, k_sb)
                for wq_sb, dst in zip(wq_tiles.pop(i), (q_sb, k_sb)):
                    for half in range(2):
                        tb = 2 * b + half
                        ps = psum.tile([128, 512], fp32, tag="mm", bufs=2,
                                       name=f"qkps_{b}_{p}_{half}")
                        for ct in range(CT):
                            nc.tensor.matmul(
                                ps[:], wq_sb[:, ct, :],
                                x_sb[ct][:, tb * 512:(tb + 1) * 512],
                                start=(ct == 0), stop=(ct == CT - 1))
                            if ct in (2, 5):
                                yield
                        nc.vector.tensor_copy(
                            dst[:, half * 512:(half + 1) * 512], ps[:])
                        yield

            def gen_v(tt, ob):
                ps = psum.tile([128, 512], fp32, tag="mm", bufs=2,
                               name=f"vg_{tt}_{ob}")
                for ct in range(CT):
                    nc.tensor.matmul(
                        ps[:], x_sb[ct][:, tt * 128:(tt + 1) * 128],
                        wv_sb[:, ct, ob * 512:(ob + 1) * 512],
                        start=(ct == 0), stop=(ct == CT - 1))
                    if ct in (2, 5):
                        yield
                nc.vector.tensor_copy(
                    v_ext[:, tt, ob * 8:(ob + 1) * 8, 0:D],
                    ps[:].rearrange("p (h d) -> p h d", d=D))

            def gen_out(tt, ob):
                ps = psum.tile([128, 512], fp32, tag="mm", bufs=2,
                               name=f"og_{tt}_{ob}")
                for j in range(CT):
                    nc.tensor.matmul(
                        ps[:], attnT[:, j, tt * 128:(tt + 1) * 128],
                        wp_sb[:, j, ob * 512:(ob + 1) * 512],
                        start=(j == 0), stop=(j == CT - 1))
                    if j in (2, 5):
                        yield
                o_sb = outpool.tile([128, 512], fp32, tag="o",
                                    name=f"og_sb_{tt}_{ob}")
                nc.vector.tensor_add(
                    o_sb[:], ps[:], bias_sb[:, ob * 512:(ob + 1) * 512])
                nc.sync.dma_start(
                    out[tt * 128:(tt + 1) * 128, ob * 512:(ob + 1) * 512],
                    o_sb[:])

            def v_chain(tt, ob):
                ps = psum.tile([128, 512], fp32, tag="mm", bufs=2,
                               name=f"vps_{tt}_{ob}")
                for ct in range(CT):
                    nc.tensor.matmul(
                        ps[:], x_sb[ct][:, tt * 128:(tt + 1) * 128],
                        wv_sb[:, ct, ob * 512:(ob + 1) * 512],
                        start=(ct == 0), stop=(ct == CT - 1))
                nc.vector.tensor_copy(
                    v_ext[:, tt, ob * 8:(ob + 1) * 8, 0:D],
                    ps[:].rearrange("p (h d) -> p h d", d=D))

            def out_chain(tt, ob):
                ps = psum.tile([128, 512], fp32, tag="mm", bufs=2,
                               name=f"ops_{tt}_{ob}")
                for j in range(CT):
                    nc.tensor.matmul(
                        ps[:], attnT[:, j, tt * 128:(tt + 1) * 128],
                        wp_sb[:, j, ob * 512:(ob + 1) * 512],
                        start=(j == 0), stop=(j == CT - 1))
                o_sb = outpool.tile([128, 512], fp32, tag="o",
                                    name=f"osb_{tt}_{ob}")
                nc.vector.tensor_add(
                    o_sb[:], ps[:], bias_sb[:, ob * 512:(ob + 1) * 512])
                nc.sync.dma_start(
                    out[tt * 128:(tt + 1) * 128, ob * 512:(ob + 1) * 512],
                    o_sb[:])

            def attn_unit(b, p, q_sb, k_sb, work, max_steps=15):
                stepped = [0]

                def maybe_fill():
                    if stepped[0] < max_steps:
                        if work.step():
                            stepped[0] += 1

                for qb in range(QB):
                    pv = [psum.tile([D + 1, 512], fp32, tag="pv", bufs=2,
                                    name=f"pv_{b}_{p}_{qb}_{h2}")
                          for h2 in range(2)]
                    e_prev = None
                    for kt in range(KT):
                        # both heads' scores into halves of one 2-bank tile,
                        # one exp over the whole [128, 1024] region
                        s_ps = psum.tile([128, 1024], fp32, tag="s", bufs=2,
                                         name=f"s_{b}_{p}_{qb}_{kt}")
                        for h2 in range(2):
                            ho = h2 * 64
                            nc.tensor.matmul(
                                s_ps[:, h2 * 512:(h2 + 1) * 512],
                                k_sb[ho:ho + 64, kt * 128:(kt + 1) * 128],
                                q_sb[ho:ho + 64, qb * 512:(qb + 1) * 512],
                                start=True, stop=True)
                        e_sb = epool.tile([128, 1024], fp16, tag="e",
                                          name=f"e_{b}_{p}_{qb}_{kt}")
                        nc.scalar.activation(
                            e_sb[:], s_ps[:], Exp,
                            bias=zero_sb[:], scale=SCALE)
                        if kt > 0:
                            for h2 in range(2):
                                nc.tensor.matmul(
                                    pv[h2][:],
                                    v_ext[:, b * KT + kt - 1, 2 * p + h2, :],
                                    e_prev[:, h2 * 512:(h2 + 1) * 512],
                                    start=(kt == 1), stop=False)
                        e_prev = e_sb
                        maybe_fill()
                    for h2 in range(2):
                        nc.tensor.matmul(
                            pv[h2][:],
                            v_ext[:, b * KT + KT - 1, 2 * p + h2, :],
                            e_prev[:, h2 * 512:(h2 + 1) * 512],
                            start=False, stop=True)
                    for h2 in range(2):
                        dn_raw = dnpool.tile([1, 512], fp32, tag="dnr",
                                             name=f"dnr_{b}_{p}_{qb}_{h2}")
                        nc.vector.tensor_copy(dn_raw[0:1, :], pv[h2][D:D + 1, :])
                        dn = dnpool.tile([1, 512], fp32, tag="dn",
                                         name=f"dn_{b}_{p}_{qb}_{h2}")
                        nc.vector.reciprocal_approx_fast(
                            dn[0:1, :], dn_raw[0:1, :])
                        bc_sb = bcpool.tile([64, 512], fp32, tag="bc",
                                            name=f"bc_{b}_{p}_{qb}_{h2}")
                        nc.gpsimd.partition_broadcast(
                            bc_sb[:], dn[0:1, :], channels=64)
                        tsl = slice(b * N + qb * 512, b * N + (qb + 1) * 512)
                        nc.vector.tensor_mul(
                            attnT[h2 * 64:h2 * 64 + 64, p, tsl],
                            pv[h2][0:D, :], bc_sb[:])
                    maybe_fill()
                    maybe_fill()

            # -------------------- emission schedule -----------------------
            class WorkQueue:
                """Ordered queue of generators advanced one yield-segment at
                a time; marks let a unit force-drain through the generator
                that produces its q/k tiles."""

                def __init__(self):
                    self.gens = []
                    self.marks = {}
                    self.i = 0

                def add(self, gen, mark=None):
                    self.gens.append(gen)
                    if mark is not None:
                        self.marks[mark] = len(self.gens) - 1

                def step(self):
                    while self.i < len(self.gens):
                        try:
                            next(self.gens[self.i])
                            return True
                        except StopIteration:
                            self.i += 1
                    return False

                def drain_mark(self, mark):
                    j = self.marks.get(mark)
                    if j is None:
                        return
                    while self.i <= j:
                        if not self.step():
                            break

                def flush(self):
                    while self.step():
                        pass

            units = [(0, p) for p in range(PAIRS)] + \
                    [(1, p) for p in range(PAIRS)]

            # prologue: weights for the first pair projections, then x
            def load_wq_for(i, stage):
                p = units[i][1]
                wq_tiles[i] = [wq_load(p, f"{stage}a"),
                               wq_load(CT + p, f"{stage}b")]

            load_wq_for(0, "p0")
            for ct in range(CT):
                nc.sync.dma_start(x_sb[ct][:], xT_r[:, ct, :])
            load_wq_for(1, "p1")
            nc.sync.dma_start(wv_sb[:], wvT_r)
            nc.sync.dma_start(wp_sb[:], wpT_r)
            nc.sync.dma_start(bias_sb[:], bias[:])
            nc.any.memset(zero_sb[:], 0.0)
            nc.any.memset(v_ext[:, :, :, D:D + 1], 1.0)

            # projections for the first two units + batch-0 v chains, inline
            load_wq_for(2, "p2")
            for _ in gen_proj_pair(0):
                pass
            load_wq_for(3, "p3")
            for _ in gen_proj_pair(1):
                pass
            for tt in range(KT):
                for ob in range(2):
                    v_chain(tt, ob)

            # queue: projection for unit i+2 inside unit i's share, then two
            # v (batch-1) or out (batch-0) chains
            work = WorkQueue()
            outs_b0 = [(tt, ob) for tt in range(KT) for ob in range(2)]
            for i in range(2, len(units)):
                work.add(gen_proj_pair(i), mark=i)
                if i - 2 < PAIRS:
                    tt = KT + (i - 2)
                    work.add(gen_v(tt, 0))
                    work.add(gen_v(tt, 1))
                else:
                    for _ in range(2):
                        if outs_b0:
                            work.add(gen_out(*outs_b0.pop(0)))
            for tt, ob in outs_b0:
                work.add(gen_out(tt, ob))

            for i, (b, p) in enumerate(units):
                if i + 4 < len(units) and (i + 4) not in wq_tiles:
                    load_wq_for(i + 4, f"u{i}")
                work.drain_mark(i)
                q_sb, k_sb = qk_ready.pop(i)
                attn_unit(b, p, q_sb, k_sb, work)
            work.flush()

            # batch-1 output projection tail
            for tt in range(KT, 2 * KT):
                for ob in range(2):
                    out_chain(tt, ob)

    nc.compile()
    return nc


def _get_nc():
    if "nc" not in _CACHE:
        _CACHE["nc"] = _build()
    return _CACHE["nc"]


def _prep_inputs(x, w_qkv, w_proj, b_proj):
    x16 = np.ascontiguousarray(x, dtype=np.float16)
    wq16 = np.asarray(w_qkv, dtype=np.float16)
    wp16 = np.asarray(w_proj, dtype=np.float16)
    wqkT_np = np.ascontiguousarray(wq16[0:2 * C].T)          # [C, 2C]
    wvT_np = np.ascontiguousarray(wq16[2 * C:3 * C].T)       # [C, C]
    wpT_np = np.ascontiguousarray(wp16.T)                    # [C, C]
    bias_np = np.ascontiguousarray(
        np.broadcast_to(np.asarray(b_proj, dtype=np.float32)[None, :], (128, C)))
    in_maps = []
    for core in range(NCORES):
        xs = x16[core * B_SH:(core + 1) * B_SH]              # [B_SH, N, C]
        xT_np = np.ascontiguousarray(xs.transpose(2, 0, 1).reshape(C, T))
        in_maps.append({
            "xT": xT_np, "wqkT": wqkT_np, "wvT": wvT_np,
            "wpT": wpT_np, "bias": bias_np,
        })
    return in_maps


def _install_ntff_hook():
    """The agent image's antenv lacks axon_hooks; synthesize it so
    run_bass_kernel_spmd(trace=True) can capture NTFF profiles."""
    import sys
    import types
    try:
        from antenv.axon_hooks import get_axon_ntff_profile_hook  # noqa: F401
        return
    except ImportError:
        pass
    import antenv
    mod = types.ModuleType("antenv.axon_hooks")
    state = {"hook": None}
    mod.set_axon_ntff_profile_hook = lambda h: state.__setitem__("hook", h)
    mod.get_axon_ntff_profile_hook = lambda: state["hook"]
    sys.modules["antenv.axon_hooks"] = mod
    antenv.axon_hooks = mod
    try:
        from trn_agent_boot.trn_boot import _ntff_profile_via_ctypes
        mod.set_axon_ntff_profile_hook(
            _ntff_profile_via_ctypes("/opt/axon/libaxon_pjrt.so"))
    except Exception as e:  # tracing degrades, run still works
        print("ntff hook install failed:", e)


def run(x, w_qkv, w_proj, b_proj, trace=False):
    """Returns (full_output [B,N,C] fp32, BassKernelResults)."""
    from concourse.bass_utils import run_bass_kernel_spmd

    if trace:
        _install_ntff_hook()
    nc = _get_nc()
    in_maps = _prep_inputs(x, w_qkv, w_proj, b_proj)
    res = run_bass_kernel_spmd(
        nc, in_maps, core_ids=list(range(NCORES)), trace=trace)
    out_full = np.concatenate(
        [r["out"].reshape(B_SH, N, C) for r in res.results], axis=0)
    return out_full.astype(np.float32), res


def kernel(x, w_qkv, w_proj, b_proj):
    out_full, _ = run(x, w_qkv, w_proj, b_proj, trace=False)
    return out_full
